# revision 1
# baseline (speedup 1.0000x reference)
"""BinaryLinear kernel for 8 Trainium2 NeuronCores.

Computes out = x @ sign(W).T + bias for x [8, 2048, 4096], W [4096, 4096],
bias [4096], all float32.

Strategy: data-parallel over the batch dim — core b handles x[b] ([2048
tokens, 4096 in]) with the full (binarized) weight matrix.

Per-core device kernel (Tile framework):
  - x[b].T is uploaded as bf16 [in=4096, tokens=2048] and kept SBUF-resident
    (16 MB), one tile per 128-row k-slice so compute starts as slices land.
  - sign(W).T is uploaded as bf16 (+-1 is exact in bf16), host-packed into
    per-out-block tiles so every weight DMA is 8KB-contiguous per partition,
    and streamed one 128-wide out-feature block per iteration.
  - TensorE computes out.T tiles: psum[o_tile 128, t 512] += wT_blk[k 128,
    o 128].T @ xT[k 128, t 512], accumulated over 32 k-tiles.
  - Phase 1 interleaves the first TWO out-blocks in one k-loop (all 8 PSUM
    banks) so the x-streaming prologue is PE-bound, not DMA-bound; phase 2
    runs the remaining 30 blocks tt-outer against the resident x.
  - ScalarE evicts PSUM -> SBUF adding the bias (per-partition AP bias).
  - Output is written as out.T [4096, 2048] f32; host transposes back.

Throwaway warm-up matmuls on a memset tile run while the first DMAs are in
flight, flipping the PE's HAM clock gate to 2.4 GHz before the real work
starts (otherwise the first ~3.4us of matmuls run at 1.2 GHz).

bf16 matmul runs at 1 cycle/row on the PE (fp32 needs 4), and rounding x to
bf16 against exact +-1 weights keeps relative error ~1.7e-3. Measured
~903-906us HW exec per core vs an 874us pure-matmul floor (4096 MMs x
~215ns): the PE runs gap-free and fully warm; remaining overhead is the
fixed engine preamble (~7us), the critical-prefix DMA before the first
matmul (~6us), the Tile exit drain (~12us), and ~5us of PE instruction-
fetch hiccups (one isolated-latency matmul every 49 — one 16KB IRAM block).
"""

import numpy as np
import ml_dtypes

B = 8
T = 2048
IN_F = 4096
OUT_F = 4096
N_CORES = 8
P = 128
KT = IN_F // P  # 32 contraction tiles
OT = OUT_F // P  # 32 out-feature tiles
TN = 512  # moving-operand free dim (one PSUM bank of f32; ISA caps mm num_elements at 512)
TT = T // TN  # 4 token slices

_compiled_nc = None


def build_program():
    import concourse.mybir as mybir
    import concourse.tile as tile
    from concourse import bacc

    nc = bacc.Bacc("TRN2", target_bir_lowering=False, debug=False)

    xT = nc.dram_tensor("xT", [IN_F, T], mybir.dt.bfloat16, kind="ExternalInput")
    # W pre-packed on host: wP[ot, p, kt*128 + o] = sign(W)[ot*128+o, kt*128+p]
    # so each per-ot block DMA is 8KB-contiguous per partition row.
    wP = nc.dram_tensor(
        "wP", [OT, P, KT, P], mybir.dt.bfloat16, kind="ExternalInput"
    )
    bv = nc.dram_tensor("biasv", [P, OT], mybir.dt.float32, kind="ExternalInput")
    oT = nc.dram_tensor("outT", [OUT_F, T], mybir.dt.float32, kind="ExternalOutput")

    xT_r = xT.ap().rearrange("(kt p) t -> p kt t", p=P)  # [128, 32, 2048]
    oT_r = oT.ap().rearrange("(ot p) t -> p ot t", p=P)  # [128, 32, 2048]

    # Phase-1 weight chunk boundaries (in k-tiles): small first chunks keep
    # the critical startup prefix small.
    CHUNKS = [(0, 4), (4, 4), (8, 8), (16, 8), (24, 8)]  # (offset, size)
    KT2CH = {}
    for ci, (off, sz) in enumerate(CHUNKS):
        for k in range(off, off + sz):
            KT2CH[k] = (ci, off)

    def evict(nc, mybir, opool, oT_r, b_sb, psum, ot, tt):
        o_sb = opool.tile([P, TN], mybir.dt.float32, name=f"o_{ot}_{tt}", tag="o")
        nc.scalar.activation(
            o_sb[:],
            psum[:],
            mybir.ActivationFunctionType.Identity,
            bias=b_sb[:, ot : ot + 1],
        )
        nc.sync.dma_start(oT_r[:, ot, tt * TN : (tt + 1) * TN], o_sb[:])

    with tile.TileContext(nc) as tc:
        with (
            tc.tile_pool(name="xpool", bufs=KT + 1) as xpool,
            tc.tile_pool(name="wcpool", bufs=2 * len(CHUNKS)) as wcpool,
            tc.tile_pool(name="wpool", bufs=3) as wpool,
            tc.tile_pool(name="bpool", bufs=2) as bpool,
            tc.tile_pool(name="opool", bufs=6) as opool,
            tc.tile_pool(name="pspool", bufs=8 * 512 // TN, space="PSUM") as pspool,
        ):
            # Warm up the PE while the first DMAs are in flight: throwaway
            # matmuls flip the HAM clock gate to 2.4 GHz (the first ~3.4us of
            # PE activity runs at 1.2 GHz otherwise) and cost nothing since
            # the PE would be idle waiting on DMA anyway.
            wu_x = bpool.tile([P, TN], mybir.dt.bfloat16, name="wu_x")
            nc.gpsimd.memset(wu_x[:], 0.0)
            wu_ps = pspool.tile([P, TN], mybir.dt.float32, name="wu_ps", tag="ps")
            for _ in range(12):
                nc.tensor.matmul(
                    wu_ps[:], wu_x[:, :P], wu_x[:], start=True, stop=True
                )

            # Phase 1: the first TWO output blocks share one k-loop (8 PSUM
            # banks) so the PE consumes each arriving x tile twice — this
            # makes the x-streaming phase PE-bound instead of DMA-bound.
            # Their weights arrive in chunks interleaved with the x stream.
            wc = {}  # (ot, c) -> tile

            def load_chunk(o2, ci):
                off, sz = CHUNKS[ci]
                w_t = wcpool.tile(
                    [P, sz, P], mybir.dt.bfloat16, name=f"wc_{o2}_{ci}", tag="wc"
                )
                nc.sync.dma_start(w_t[:], wP.ap()[o2][:, off : off + sz, :])
                wc[(o2, ci)] = w_t

            def load_chunk_pair(c):
                for o2 in range(2):
                    load_chunk(o2, c)

            def load_x(kt, eng=None):
                x_t = xpool.tile([P, T], mybir.dt.bfloat16, name=f"x_{kt}", tag="x")
                (eng or nc.sync).dma_start(x_t[:], xT_r[:, kt, :])
                x_tiles[kt] = x_t

            x_tiles = {}
            # x0 is split in half so the very first matmuls wait on 256KB,
            # not 512KB.
            x0_halves = []
            HALF = T // 2

            def load_x0_half(h):
                x_h = xpool.tile(
                    [P, HALF], mybir.dt.bfloat16, name=f"x_0{'ab'[h]}", tag="x"
                )
                nc.sync.dma_start(x_h[:], xT_r[:, 0, h * HALF : (h + 1) * HALF])
                x0_halves.append(x_h)

            def x_slice(kt, tt):
                if kt == 0:
                    h, tt_in = divmod(tt, HALF // TN)
                    return x0_halves[h][:, tt_in * TN : (tt_in + 1) * TN]
                return x_tiles[kt][:, tt * TN : (tt + 1) * TN]

            # Interleave chunk-pair and x-tile DMAs in consumption order.
            # The very first matmul needs only wc(0,0) + x0a, so issue those
            # two first.
            load_chunk(0, 0)
            load_x0_half(0)
            load_chunk(1, 0)
            load_x0_half(1)
            load_chunk_pair(1)
            for kt in range(1, 6):
                load_x(kt)
            # Bias is tiny but descriptor-heavy; keep it off the critical
            # startup path (first needed at the first eviction, ~60us in).
            b_sb = bpool.tile([P, OT], mybir.dt.float32, name="b_sb")
            nc.sync.dma_start(b_sb[:], bv.ap())
            load_chunk_pair(2)
            for kt in range(6, 14):
                load_x(kt)
            load_chunk_pair(3)
            for kt in range(14, 22):
                load_x(kt)
            load_chunk_pair(4)
            for kt in range(22, KT):
                load_x(kt)

            psums1 = [
                [
                    pspool.tile([P, TN], mybir.dt.float32, name=f"ps_{o2}_{tt}", tag="ps")
                    for tt in range(TT)
                ]
                for o2 in range(2)
            ]
            # kt=0 consumes x0a for both blocks before touching x0b, matching
            # DMA arrival order.
            for o2, tt in [(0, 0), (0, 1), (1, 0), (1, 1), (0, 2), (0, 3), (1, 2), (1, 3)]:
                nc.tensor.matmul(
                    psums1[o2][tt][:],
                    wc[(o2, 0)][:, 0, :],
                    x_slice(0, tt),
                    start=True,
                    stop=False,
                )
            for kt in range(1, KT):
                for o2 in range(2):
                    ci, off = KT2CH[kt]
                    lhsT = wc[(o2, ci)][:, kt - off, :]
                    for tt in range(TT):
                        nc.tensor.matmul(
                            psums1[o2][tt][:],
                            lhsT,
                            x_slice(kt, tt),
                            start=False,
                            stop=(kt == KT - 1),
                        )
            for o2 in range(2):
                for tt in range(TT):
                    evict(nc, mybir, opool, oT_r, b_sb, psums1[o2][tt], o2, tt)

            # Phase 2: remaining blocks against the resident x.
            for ot in range(2, OT):
                w_sb = wpool.tile(
                    [P, KT, P], mybir.dt.bfloat16, name=f"w_{ot}", tag="w"
                )
                nc.sync.dma_start(w_sb[:], wP.ap()[ot])

                # tt-outer: each PSUM bank finishes its 32-matmul group in a
                # burst and evicts while the next bank accumulates, so
                # evictions never pile up after the block's last matmul.
                for tt in range(TT):
                    last_group = ot == OT - 1 and tt == TT - 1
                    if not last_group:
                        psum = pspool.tile(
                            [P, TN], mybir.dt.float32, name=f"ps_{ot}_{tt}", tag="ps"
                        )
                        for kt in range(KT):
                            nc.tensor.matmul(
                                psum[:],
                                w_sb[:, kt, :],
                                x_slice(kt, tt),
                                start=(kt == 0),
                                stop=(kt == KT - 1),
                            )
                        evict(nc, mybir, opool, oT_r, b_sb, psum, ot, tt)
                    else:
                        # The kernel's very last group is split into two
                        # half-width groups run sequentially, so the final
                        # evict+DMA chain (which nothing can overlap) covers
                        # 128KB instead of 256KB.
                        HN = TN // 2
                        for h in range(2):
                            psum = pspool.tile(
                                [P, HN],
                                mybir.dt.float32,
                                name=f"ps_{ot}_{tt}_{h}",
                                tag="ps",
                            )
                            for kt in range(KT):
                                if kt == 0:
                                    rhs = x0_halves[1][
                                        :, HALF - TN + h * HN : HALF - TN + (h + 1) * HN
                                    ]
                                else:
                                    rhs = x_tiles[kt][
                                        :, tt * TN + h * HN : tt * TN + (h + 1) * HN
                                    ]
                                nc.tensor.matmul(
                                    psum[:],
                                    w_sb[:, kt, :],
                                    rhs,
                                    start=(kt == 0),
                                    stop=(kt == KT - 1),
                                )
                            o_sb = opool.tile(
                                [P, HN], mybir.dt.float32, name=f"o_{ot}_{tt}_{h}", tag="o"
                            )
                            nc.scalar.activation(
                                o_sb[:],
                                psum[:],
                                mybir.ActivationFunctionType.Identity,
                                bias=b_sb[:, ot : ot + 1],
                            )
                            nc.sync.dma_start(
                                oT_r[:, ot, tt * TN + h * HN : tt * TN + (h + 1) * HN],
                                o_sb[:],
                            )

    nc.compile()
    return nc


def prepare_inputs(x, weight, bias):
    """Host-side layout prep: transpose + cast per-core shards."""
    bf16 = ml_dtypes.bfloat16
    x = np.asarray(x, dtype=np.float32)
    weight = np.asarray(weight, dtype=np.float32)
    bias = np.asarray(bias, dtype=np.float32)
    w_bin = np.where(weight >= 0, np.float32(1.0), np.float32(-1.0))
    # wP[ot, p, kt, o] = sign(W)[ot*128+o, kt*128+p] — per-ot weight blocks,
    # contiguous along (kt, o) so block DMAs are 8KB-contiguous per partition.
    wP_np = np.ascontiguousarray(
        w_bin.reshape(OT, P, KT, P).transpose(0, 3, 2, 1)
    ).astype(bf16)
    bv_np = np.ascontiguousarray(
        np.asarray(bias, dtype=np.float32).reshape(OT, P).T
    )  # [P, OT]; bias[o] at [o % 128, o // 128]
    in_maps = []
    for b in range(B):
        xT_np = np.ascontiguousarray(x[b].T).astype(bf16)  # [in, tokens]
        in_maps.append({"xT": xT_np, "wP": wP_np, "biasv": bv_np})
    return in_maps


def _ensure_ntff_hook_shim():
    """bass_utils' trace path imports antenv.axon_hooks, which some images
    lack; provide a working shim (or a None hook) so tracing never crashes."""
    import sys
    import types

    try:
        import antenv.axon_hooks  # noqa: F401

        return
    except ImportError:
        pass
    hook = None
    try:
        from trn_agent_boot.trn_boot import _ntff_profile_via_ctypes

        hook = _ntff_profile_via_ctypes("/opt/axon/libaxon_pjrt.so")
    except Exception:
        pass
    mod = types.ModuleType("antenv.axon_hooks")
    mod.get_axon_ntff_profile_hook = lambda: hook
    mod.set_axon_ntff_profile_hook = lambda h: None
    sys.modules["antenv.axon_hooks"] = mod
    try:
        import antenv

        antenv.axon_hooks = mod
    except ImportError:
        pass


def run(in_maps, trace=False, **kwargs):
    global _compiled_nc
    if _compiled_nc is None:
        _compiled_nc = build_program()
    _ensure_ntff_hook_shim()
    from concourse.bass_utils import run_bass_kernel_spmd

    return run_bass_kernel_spmd(
        _compiled_nc, in_maps, list(range(N_CORES)), trace=trace, **kwargs
    )


def kernel(x, weight, bias):
    res = run(prepare_inputs(x, weight, bias))
    out = np.empty((B, T, OUT_F), dtype=np.float32)
    for b in range(B):
        out[b] = res.results[b]["outT"].T
    return out



# revision 2
# speedup vs baseline: 1.3209x; 1.3209x over previous
"""BinaryLinear kernel for 8 Trainium2 NeuronCores.

Computes out = x @ sign(W).T + bias for x [8, 2048, 4096], W [4096, 4096],
bias [4096], all float32.

Strategy: data-parallel over the batch dim — core b handles x[b] ([2048
tokens, 4096 in]) with the full (binarized) weight matrix.

Per-core device kernel (Tile framework) — MIXED-PRECISION contraction:
  - The 32 contraction k-tiles are split KB=16 in bf16 + 8 fp8(e4m3)
    DoubleRow pairs. sign(W) is exact in both dtypes; only x is rounded.
    fp8 DoubleRow matmuls process TWO k-tiles (256-deep contraction) per
    instruction at the same ~216ns as one bf16 k-tile (2x PE throughput,
    measured: LDWEIGHTS fully hidden at FD=512). Per (out-block, token
    slice): 16 bf16 matmuls + 8 DoubleRow matmuls = ~5.2us vs 6.9us
    all-bf16. Quantizing half of x to e4m3 gives deterministic rel err
    1.88e-2 (vs 1.7e-3 all-bf16), under the 2e-2 gate.
  - x.T is uploaded split: bf16 rows [0, 2048) and fp8 rows [2048, 4096),
    kept SBUF-resident (12 MB), one tile per k-slice (fp8 tiles hold a
    DoubleRow pair [128, 2, 2048]).
  - Weights are host-packed per out-block into bf16 [128, KB, 128] and
    fp8 [128, NP, 2, 128] blocks so every weight DMA is contiguous per
    partition row.
  - TensorE computes out.T tiles: psum[o 128, t 512] accumulates KB bf16
    matmuls + NP DoubleRow matmuls (mixed-dtype PSUM groups verified).
  - Phase 1 interleaves the first TWO out-blocks in one k-loop (all 8
    PSUM banks) so the x-streaming prologue stays PE-bound; phase 2 runs
    the remaining 30 blocks tt-outer against the resident x.
  - ScalarE evicts PSUM -> SBUF adding the bias; output is written as
    out.T [4096, 2048] f32; host transposes back.

Throwaway warm-up matmuls on a memset tile run while the first DMAs are
in flight, flipping the PE's HAM clock gate to 2.4 GHz before the real
work starts.
"""

import numpy as np
import ml_dtypes

B = 8
T = 2048
IN_F = 4096
OUT_F = 4096
N_CORES = 8
P = 128
KT = IN_F // P  # 32 contraction tiles
OT = OUT_F // P  # 32 out-feature tiles
TN = 512  # moving-operand free dim (one PSUM bank of f32)
TT = T // TN  # 4 token slices

KB = 16  # bf16 k-tiles (k-tiles 0..KB-1)
KF = KT - KB  # fp8 k-tiles
NP = KF // 2  # fp8 DoubleRow pairs

_compiled_nc = None


def _chunks(n, first=4, rest=8):
    """Chunk 0..n into (offset, size) runs: two small leading chunks keep
    the critical startup prefix small."""
    out = []
    off = 0
    while off < n:
        sz = min(first if off < 2 * first else rest, n - off)
        out.append((off, sz))
        off += sz
    return out


def build_program():
    import concourse.mybir as mybir
    import concourse.tile as tile
    from concourse import bacc

    DR = mybir.MatmulPerfMode.DoubleRow

    nc = bacc.Bacc("TRN2", target_bir_lowering=False, debug=False)

    xTb = nc.dram_tensor("xTb", [KB * P, T], mybir.dt.bfloat16, kind="ExternalInput")
    xTf = nc.dram_tensor("xTf", [KF * P, T], mybir.dt.float8e4, kind="ExternalInput")
    # Host-packed weights: wPb[ot, p, kt, o] = sign(W)[ot*128+o, kt*128+p],
    # wPf[ot, p, j, i, o] = sign(W)[ot*128+o, (KB+2j+i)*128+p].
    wPb = nc.dram_tensor("wPb", [OT, P, KB, P], mybir.dt.bfloat16, kind="ExternalInput")
    wPf = nc.dram_tensor(
        "wPf", [OT, P, NP, 2, P], mybir.dt.float8e4, kind="ExternalInput"
    )
    bv = nc.dram_tensor("biasv", [P, OT], mybir.dt.float32, kind="ExternalInput")
    oT = nc.dram_tensor("outT", [OUT_F, T], mybir.dt.float32, kind="ExternalOutput")

    xb_r = xTb.ap().rearrange("(kt p) t -> p kt t", p=P)  # [128, KB, 2048]
    xf_r = xTf.ap().rearrange("(kt p) t -> p kt t", p=P)  # [128, KF, 2048]
    oT_r = oT.ap().rearrange("(ot p) t -> p ot t", p=P)  # [128, 32, 2048]

    CHB = _chunks(KB)  # bf16 phase-1 weight chunks (in k-tiles)
    CHF = _chunks(NP, first=4, rest=4)  # fp8 phase-1 weight chunks (in pairs)
    KT2CHB = {}
    for ci, (off, sz) in enumerate(CHB):
        for k in range(off, off + sz):
            KT2CHB[k] = (ci, off)
    J2CHF = {}
    for ci, (off, sz) in enumerate(CHF):
        for j in range(off, off + sz):
            J2CHF[j] = (ci, off)

    def evict(nc, mybir, opool, oT_r, b_sb, psum, ot, tt):
        o_sb = opool.tile([P, TN], mybir.dt.float32, name=f"o_{ot}_{tt}", tag="o")
        nc.scalar.activation(
            o_sb[:],
            psum[:],
            mybir.ActivationFunctionType.Identity,
            bias=b_sb[:, ot : ot + 1],
        )
        nc.sync.dma_start(oT_r[:, ot, tt * TN : (tt + 1) * TN], o_sb[:])

    with tile.TileContext(nc) as tc:
        with (
            tc.tile_pool(name="xpool", bufs=KB + 1) as xpool,
            tc.tile_pool(name="xfpool", bufs=NP) as xfpool,
            tc.tile_pool(name="wcbpool", bufs=2 * len(CHB)) as wcbpool,
            tc.tile_pool(name="wcfpool", bufs=2 * len(CHF)) as wcfpool,
            tc.tile_pool(name="wbpool", bufs=3) as wbpool,
            tc.tile_pool(name="wfpool", bufs=3) as wfpool,
            tc.tile_pool(name="bpool", bufs=2) as bpool,
            tc.tile_pool(name="opool", bufs=6) as opool,
            tc.tile_pool(name="pspool", bufs=8 * 512 // TN, space="PSUM") as pspool,
        ):
            # Warm up the PE while the first DMAs are in flight (HAM clock
            # gate -> 2.4 GHz).
            wu_x = bpool.tile([P, TN], mybir.dt.bfloat16, name="wu_x")
            nc.gpsimd.memset(wu_x[:], 0.0)
            wu_ps = pspool.tile([P, TN], mybir.dt.float32, name="wu_ps", tag="ps")
            for _ in range(12):
                nc.tensor.matmul(
                    wu_ps[:], wu_x[:, :P], wu_x[:], start=True, stop=True
                )

            # Phase 1: the first TWO output blocks share one k-loop (8 PSUM
            # banks) so the PE consumes each arriving x tile twice — this
            # keeps the x-streaming phase PE-bound instead of DMA-bound.
            wcB = {}  # (o2, ci) -> bf16 chunk tile
            wcF = {}  # (o2, ci) -> fp8 chunk tile

            def load_chunk_b(o2, ci):
                off, sz = CHB[ci]
                w_t = wcbpool.tile(
                    [P, sz, P], mybir.dt.bfloat16, name=f"wcb_{o2}_{ci}", tag="wcb"
                )
                nc.sync.dma_start(w_t[:], wPb.ap()[o2][:, off : off + sz, :])
                wcB[(o2, ci)] = w_t

            def load_chunk_f(o2, ci):
                off, sz = CHF[ci]
                w_t = wcfpool.tile(
                    [P, sz, 2, P], mybir.dt.float8e4, name=f"wcf_{o2}_{ci}", tag="wcf"
                )
                nc.sync.dma_start(w_t[:], wPf.ap()[o2][:, off : off + sz, :, :])
                wcF[(o2, ci)] = w_t

            def load_x(kt):
                x_t = xpool.tile([P, T], mybir.dt.bfloat16, name=f"x_{kt}", tag="x")
                nc.sync.dma_start(x_t[:], xb_r[:, kt, :])
                x_tiles[kt] = x_t

            def load_xf(j):
                x_t = xfpool.tile(
                    [P, 2, T], mybir.dt.float8e4, name=f"xf_{j}", tag="xf"
                )
                nc.sync.dma_start(x_t[:], xf_r[:, 2 * j : 2 * j + 2, :])
                xf_tiles[j] = x_t

            x_tiles = {}
            xf_tiles = {}
            # x0 is split in half so the very first matmuls wait on 256KB,
            # not 512KB.
            x0_halves = []
            HALF = T // 2

            def load_x0_half(h):
                x_h = xpool.tile(
                    [P, HALF], mybir.dt.bfloat16, name=f"x_0{'ab'[h]}", tag="x"
                )
                nc.sync.dma_start(x_h[:], xb_r[:, 0, h * HALF : (h + 1) * HALF])
                x0_halves.append(x_h)

            def x_slice(kt, tt):
                if kt == 0:
                    h, tt_in = divmod(tt, HALF // TN)
                    return x0_halves[h][:, tt_in * TN : (tt_in + 1) * TN]
                return x_tiles[kt][:, tt * TN : (tt + 1) * TN]

            # Interleave chunk and x DMAs in consumption order. The very
            # first matmul needs only wcB(0,0) + x0a, so issue those first.
            load_chunk_b(0, 0)
            load_x0_half(0)
            load_chunk_b(1, 0)
            load_x0_half(1)
            for o2 in range(2):
                load_chunk_b(o2, 1)
            for kt in range(1, 6):
                load_x(kt)
            # Bias is tiny but descriptor-heavy; keep it off the critical
            # startup path.
            b_sb = bpool.tile([P, OT], mybir.dt.float32, name="b_sb")
            nc.sync.dma_start(b_sb[:], bv.ap())
            for o2 in range(2):
                load_chunk_b(o2, 2)
            for kt in range(6, KB):
                load_x(kt)
            for o2 in range(2):
                load_chunk_f(o2, 0)
            for j in range(0, min(4, NP)):
                load_xf(j)
            for o2 in range(2):
                for ci in range(1, len(CHF)):
                    load_chunk_f(o2, ci)
            for j in range(4, NP):
                load_xf(j)

            psums1 = [
                [
                    pspool.tile([P, TN], mybir.dt.float32, name=f"ps_{o2}_{tt}", tag="ps")
                    for tt in range(TT)
                ]
                for o2 in range(2)
            ]
            # kt=0 consumes x0a for both blocks before touching x0b, matching
            # DMA arrival order.
            for o2, tt in [(0, 0), (0, 1), (1, 0), (1, 1), (0, 2), (0, 3), (1, 2), (1, 3)]:
                nc.tensor.matmul(
                    psums1[o2][tt][:],
                    wcB[(o2, 0)][:, 0, :],
                    x_slice(0, tt),
                    start=True,
                    stop=False,
                )
            for kt in range(1, KB):
                for o2 in range(2):
                    ci, off = KT2CHB[kt]
                    lhsT = wcB[(o2, ci)][:, kt - off, :]
                    for tt in range(TT):
                        nc.tensor.matmul(
                            psums1[o2][tt][:],
                            lhsT,
                            x_slice(kt, tt),
                            start=False,
                            stop=False,
                        )
            for j in range(NP):
                for o2 in range(2):
                    ci, off = J2CHF[j]
                    lhsT = wcF[(o2, ci)][:, j - off, :, :]
                    for tt in range(TT):
                        nc.tensor.matmul(
                            psums1[o2][tt][:],
                            lhsT,
                            xf_tiles[j][:, :, tt * TN : (tt + 1) * TN],
                            start=False,
                            stop=(j == NP - 1),
                            perf_mode=DR,
                        )
            for o2 in range(2):
                for tt in range(TT):
                    evict(nc, mybir, opool, oT_r, b_sb, psums1[o2][tt], o2, tt)

            # Phase 2: remaining blocks against the resident x.
            for ot in range(2, OT):
                wb_sb = wbpool.tile(
                    [P, KB, P], mybir.dt.bfloat16, name=f"wb_{ot}", tag="wb"
                )
                nc.sync.dma_start(wb_sb[:], wPb.ap()[ot])
                wf_sb = wfpool.tile(
                    [P, NP, 2, P], mybir.dt.float8e4, name=f"wf_{ot}", tag="wf"
                )
                nc.sync.dma_start(wf_sb[:], wPf.ap()[ot])

                # tt-outer: each PSUM bank finishes its matmul group in a
                # burst and evicts while the next bank accumulates.
                for tt in range(TT):
                    last_group = ot == OT - 1 and tt == TT - 1
                    if not last_group:
                        psum = pspool.tile(
                            [P, TN], mybir.dt.float32, name=f"ps_{ot}_{tt}", tag="ps"
                        )
                        for kt in range(KB):
                            nc.tensor.matmul(
                                psum[:],
                                wb_sb[:, kt, :],
                                x_slice(kt, tt),
                                start=(kt == 0),
                                stop=False,
                            )
                        for j in range(NP):
                            nc.tensor.matmul(
                                psum[:],
                                wf_sb[:, j, :, :],
                                xf_tiles[j][:, :, tt * TN : (tt + 1) * TN],
                                start=False,
                                stop=(j == NP - 1),
                                perf_mode=DR,
                            )
                        evict(nc, mybir, opool, oT_r, b_sb, psum, ot, tt)
                    else:
                        # The kernel's very last group is split into two
                        # half-width groups run sequentially, so the final
                        # evict+DMA chain (which nothing can overlap) covers
                        # 128KB instead of 256KB.
                        HN = TN // 2
                        for h in range(2):
                            psum = pspool.tile(
                                [P, HN],
                                mybir.dt.float32,
                                name=f"ps_{ot}_{tt}_{h}",
                                tag="ps",
                            )
                            lo = tt * TN + h * HN
                            for kt in range(KB):
                                if kt == 0:
                                    rhs = x0_halves[1][
                                        :, HALF - TN + h * HN : HALF - TN + (h + 1) * HN
                                    ]
                                else:
                                    rhs = x_tiles[kt][:, lo : lo + HN]
                                nc.tensor.matmul(
                                    psum[:],
                                    wb_sb[:, kt, :],
                                    rhs,
                                    start=(kt == 0),
                                    stop=False,
                                )
                            for j in range(NP):
                                nc.tensor.matmul(
                                    psum[:],
                                    wf_sb[:, j, :, :],
                                    xf_tiles[j][:, :, lo : lo + HN],
                                    start=False,
                                    stop=(j == NP - 1),
                                    perf_mode=DR,
                                )
                            o_sb = opool.tile(
                                [P, HN], mybir.dt.float32, name=f"o_{ot}_{tt}_{h}", tag="o"
                            )
                            nc.scalar.activation(
                                o_sb[:],
                                psum[:],
                                mybir.ActivationFunctionType.Identity,
                                bias=b_sb[:, ot : ot + 1],
                            )
                            nc.sync.dma_start(
                                oT_r[:, ot, lo : lo + HN],
                                o_sb[:],
                            )

    nc.compile()
    return nc


def prepare_inputs(x, weight, bias):
    """Host-side layout prep: transpose + cast per-core shards."""
    bf16 = ml_dtypes.bfloat16
    fp8 = ml_dtypes.float8_e4m3
    x = np.asarray(x, dtype=np.float32)
    weight = np.asarray(weight, dtype=np.float32)
    bias = np.asarray(bias, dtype=np.float32)
    w_bin = np.where(weight >= 0, np.float32(1.0), np.float32(-1.0))
    # wP[ot, p, kt, o] = sign(W)[ot*128+o, kt*128+p] — per-ot weight blocks,
    # contiguous along (kt, o) so block DMAs are contiguous per partition.
    wP_np = np.ascontiguousarray(w_bin.reshape(OT, P, KT, P).transpose(0, 3, 2, 1))
    wPb_np = np.ascontiguousarray(wP_np[:, :, :KB, :]).astype(bf16)
    wPf_np = np.ascontiguousarray(wP_np[:, :, KB:, :]).astype(fp8).reshape(
        OT, P, NP, 2, P
    )
    bv_np = np.ascontiguousarray(
        np.asarray(bias, dtype=np.float32).reshape(OT, P).T
    )  # [P, OT]; bias[o] at [o % 128, o // 128]
    in_maps = []
    for b in range(B):
        xT_np = np.ascontiguousarray(x[b].T)  # [in, tokens]
        in_maps.append(
            {
                "xTb": xT_np[: KB * P].astype(bf16),
                "xTf": xT_np[KB * P :].astype(fp8),
                "wPb": wPb_np,
                "wPf": wPf_np,
                "biasv": bv_np,
            }
        )
    return in_maps


def _ensure_ntff_hook_shim():
    """bass_utils' trace path imports antenv.axon_hooks, which some images
    lack; provide a working shim (or a None hook) so tracing never crashes."""
    import sys
    import types

    try:
        import antenv.axon_hooks  # noqa: F401

        return
    except ImportError:
        pass
    hook = None
    try:
        from trn_agent_boot.trn_boot import _ntff_profile_via_ctypes

        hook = _ntff_profile_via_ctypes("/opt/axon/libaxon_pjrt.so")
    except Exception:
        pass
    mod = types.ModuleType("antenv.axon_hooks")
    mod.get_axon_ntff_profile_hook = lambda: hook
    mod.set_axon_ntff_profile_hook = lambda h: None
    sys.modules["antenv.axon_hooks"] = mod
    try:
        import antenv

        antenv.axon_hooks = mod
    except ImportError:
        pass


def run(in_maps, trace=False, **kwargs):
    global _compiled_nc
    if _compiled_nc is None:
        _compiled_nc = build_program()
    _ensure_ntff_hook_shim()
    from concourse.bass_utils import run_bass_kernel_spmd

    return run_bass_kernel_spmd(
        _compiled_nc, in_maps, list(range(N_CORES)), trace=trace, **kwargs
    )


def kernel(x, weight, bias):
    res = run(prepare_inputs(x, weight, bias))
    out = np.empty((B, T, OUT_F), dtype=np.float32)
    for b in range(B):
        out[b] = res.results[b]["outT"].T
    return out


# revision 3
# speedup vs baseline: 1.3271x; 1.0047x over previous
"""BinaryLinear kernel for 8 Trainium2 NeuronCores.

Computes out = x @ sign(W).T + bias for x [8, 2048, 4096], W [4096, 4096],
bias [4096], all float32.

Strategy: data-parallel over the batch dim — core b handles x[b] ([2048
tokens, 4096 in]) with the full (binarized) weight matrix.

Per-core device kernel (Tile framework) — MIXED-PRECISION contraction:
  - The 32 contraction k-tiles are split KB=16 in bf16 + 8 fp8(e4m3)
    DoubleRow pairs. sign(W) is exact in both dtypes; only x is rounded.
    fp8 DoubleRow matmuls process TWO k-tiles (256-deep contraction) per
    instruction at the same ~216ns as one bf16 k-tile (2x PE throughput;
    LDWEIGHTS fully hidden at FD=512). Per (out-block, token slice):
    16 bf16 + 8 DoubleRow matmuls = ~5.2us vs 6.9us all-bf16. Quantizing
    half of x to e4m3 gives deterministic rel err 1.88e-2 (vs 1.7e-3
    all-bf16), under the 2e-2 gate.
  - Each block runs its bf16 matmuls for all 4 token slices (kt-outer,
    tt-inner over 4 PSUM banks), then all DoubleRow matmuls — 2 PE
    perf-mode transitions per block instead of per token slice.
  - x.T is uploaded split: bf16 rows [0, 2048) and fp8 rows [2048, 4096),
    kept SBUF-resident (12 MB) as half-token tiles ([128, 1024] /
    [128, 2, 1024]) so phase 1 can run on the first token halves while
    the second halves stream in.
  - Phase 1 interleaves the first THREE out-blocks over two half-token
    passes (6 PSUM banks each) so the x-streaming prologue needs only
    ~230 GB/s and stays PE-bound; phase 2 runs the remaining 29 blocks
    against the resident x.
  - Weights are host-packed per out-block into bf16 [128, KB, 128] and
    fp8 [128, NP, 2, 128] blocks so every weight DMA is contiguous per
    partition row; phase-1 blocks stream their weights in k-chunks.
  - ScalarE evicts PSUM -> SBUF adding the bias (per-partition AP bias).
  - Output is written as out.T [4096, 2048] f32; host transposes back.

Throwaway warm-up matmuls on a memset tile run while the first DMAs are
in flight, flipping the PE's HAM clock gate to 2.4 GHz before the real
work starts.
"""

import numpy as np
import ml_dtypes

B = 8
T = 2048
IN_F = 4096
OUT_F = 4096
N_CORES = 8
P = 128
KT = IN_F // P  # 32 contraction tiles
OT = OUT_F // P  # 32 out-feature tiles
TN = 512  # moving-operand free dim (one PSUM bank of f32)
TT = T // TN  # 4 token slices
TH = T // 2  # half-token span (phase-1 pass granularity)

KB = 16  # bf16 k-tiles (k-tiles 0..KB-1)
KF = KT - KB  # fp8 k-tiles
NP = KF // 2  # fp8 DoubleRow pairs
NB1 = 3  # phase-1 interleaved out-blocks

_compiled_nc = None


def _chunks(n, first=4, rest=8):
    """Chunk 0..n into (offset, size) runs: small leading chunks keep the
    critical startup prefix small."""
    out = []
    off = 0
    while off < n:
        sz = min(first if off < 2 * first else rest, n - off)
        out.append((off, sz))
        off += sz
    return out


def build_program():
    import concourse.mybir as mybir
    import concourse.tile as tile
    from concourse import bacc

    DR = mybir.MatmulPerfMode.DoubleRow

    nc = bacc.Bacc("TRN2", target_bir_lowering=False, debug=False)

    xTb = nc.dram_tensor("xTb", [KB * P, T], mybir.dt.bfloat16, kind="ExternalInput")
    xTf = nc.dram_tensor("xTf", [KF * P, T], mybir.dt.float8e4, kind="ExternalInput")
    # Host-packed weights: wPb[ot, p, kt, o] = sign(W)[ot*128+o, kt*128+p],
    # wPf[ot, p, j, i, o] = sign(W)[ot*128+o, (KB+2j+i)*128+p].
    wPb = nc.dram_tensor("wPb", [OT, P, KB, P], mybir.dt.bfloat16, kind="ExternalInput")
    wPf = nc.dram_tensor(
        "wPf", [OT, P, NP, 2, P], mybir.dt.float8e4, kind="ExternalInput"
    )
    bv = nc.dram_tensor("biasv", [P, OT], mybir.dt.float32, kind="ExternalInput")
    oT = nc.dram_tensor("outT", [OUT_F, T], mybir.dt.float32, kind="ExternalOutput")

    xb_r = xTb.ap().rearrange("(kt p) t -> p kt t", p=P)  # [128, KB, 2048]
    xf_r = xTf.ap().rearrange("(kt p) t -> p kt t", p=P)  # [128, KF, 2048]
    oT_r = oT.ap().rearrange("(ot p) t -> p ot t", p=P)  # [128, 32, 2048]

    CHB = _chunks(KB)  # bf16 phase-1 weight chunks (in k-tiles)
    CHF = _chunks(NP, first=4, rest=4)  # fp8 phase-1 weight chunks (in pairs)
    KT2CHB = {}
    for ci, (off, sz) in enumerate(CHB):
        for k in range(off, off + sz):
            KT2CHB[k] = (ci, off)
    J2CHF = {}
    for ci, (off, sz) in enumerate(CHF):
        for j in range(off, off + sz):
            J2CHF[j] = (ci, off)

    def evict(psum, ot, tt, lo=None, n=TN):
        if lo is None:
            lo = tt * TN
        o_sb = opool.tile([P, n], mybir.dt.float32, name=f"o_{ot}_{lo}", tag="o")
        nc.scalar.activation(
            o_sb[:],
            psum[:],
            mybir.ActivationFunctionType.Identity,
            bias=b_sb[:, ot : ot + 1],
        )
        nc.sync.dma_start(oT_r[:, ot, lo : lo + n], o_sb[:])

    with tile.TileContext(nc) as tc:
        with (
            tc.tile_pool(name="xpool", bufs=2 * KB + 1) as xpool,
            tc.tile_pool(name="xfpool", bufs=2 * NP) as xfpool,
            tc.tile_pool(name="wcbpool", bufs=NB1 * len(CHB)) as wcbpool,
            tc.tile_pool(name="wcfpool", bufs=NB1 * len(CHF)) as wcfpool,
            tc.tile_pool(name="wbpool", bufs=3) as wbpool,
            tc.tile_pool(name="wfpool", bufs=3) as wfpool,
            tc.tile_pool(name="bpool", bufs=2) as bpool,
            tc.tile_pool(name="opool", bufs=6) as opool,
            tc.tile_pool(name="pspool", bufs=8 * 512 // TN, space="PSUM") as pspool,
        ):
            # Warm up the PE while the first DMAs are in flight (HAM clock
            # gate -> 2.4 GHz).
            wu_x = bpool.tile([P, TN], mybir.dt.bfloat16, name="wu_x")
            nc.gpsimd.memset(wu_x[:], 0.0)
            wu_ps = pspool.tile([P, TN], mybir.dt.float32, name="wu_ps", tag="ps")
            for _ in range(12):
                nc.tensor.matmul(
                    wu_ps[:], wu_x[:, :P], wu_x[:], start=True, stop=True
                )

            # ---- phase-1 weight chunks (blocks 0..NB1-1) ----
            wcB = {}  # (b3, ci) -> bf16 chunk tile
            wcF = {}  # (b3, ci) -> fp8 chunk tile

            def load_chunk_b(b3, ci):
                off, sz = CHB[ci]
                w_t = wcbpool.tile(
                    [P, sz, P], mybir.dt.bfloat16, name=f"wcb_{b3}_{ci}", tag="wcb"
                )
                nc.sync.dma_start(w_t[:], wPb.ap()[b3][:, off : off + sz, :])
                wcB[(b3, ci)] = w_t

            def load_chunk_f(b3, ci):
                off, sz = CHF[ci]
                w_t = wcfpool.tile(
                    [P, sz, 2, P], mybir.dt.float8e4, name=f"wcf_{b3}_{ci}", tag="wcf"
                )
                nc.sync.dma_start(w_t[:], wPf.ap()[b3][:, off : off + sz, :, :])
                wcF[(b3, ci)] = w_t

            # ---- x tiles: half-token granularity (A = tokens [0,1024),
            # B = tokens [1024,2048)) so phase 1 runs on A while B streams ----
            xA = {}
            xB = {}
            xfA = {}
            xfB = {}

            def load_xh(kt, half):
                store = xA if half == 0 else xB
                x_t = xpool.tile(
                    [P, TH], mybir.dt.bfloat16, name=f"x_{kt}_{half}", tag="x"
                )
                nc.sync.dma_start(
                    x_t[:], xb_r[:, kt, half * TH : (half + 1) * TH]
                )
                store[kt] = x_t

            def load_xfh(j, half):
                store = xfA if half == 0 else xfB
                x_t = xfpool.tile(
                    [P, 2, TH], mybir.dt.float8e4, name=f"xf_{j}_{half}", tag="xf"
                )
                nc.sync.dma_start(
                    x_t[:], xf_r[:, 2 * j : 2 * j + 2, half * TH : (half + 1) * TH]
                )
                store[j] = x_t

            # x(kt=0, tokens [0,512)) is its own tile so the very first
            # matmuls wait on 128KB, not 256KB.
            x0q = []

            def load_x0q(q):
                x_t = xpool.tile([P, TN], mybir.dt.bfloat16, name=f"x_0q{q}", tag="x")
                nc.sync.dma_start(x_t[:], xb_r[:, 0, q * TN : (q + 1) * TN])
                x0q.append(x_t)

            def x_slice(kt, tt):
                if kt == 0 and tt < 2:
                    return x0q[tt][:]
                half, tl = divmod(tt * TN, TH)
                src = xA if half == 0 else xB
                return src[kt][:, tl : tl + TN]

            def xf_slice(j, tt):
                half, tl = divmod(tt * TN, TH)
                src = xfA if half == 0 else xfB
                return src[j][:, :, tl : tl + TN]

            # ---- DMA issue order: by first-use time ----
            load_chunk_b(0, 0)
            load_x0q(0)
            load_chunk_b(1, 0)
            load_x0q(1)
            load_chunk_b(2, 0)
            for kt in range(1, 4):
                load_xh(kt, 0)
            for b3 in range(NB1):
                load_chunk_b(b3, 1)
            for kt in range(4, 8):
                load_xh(kt, 0)
            for b3 in range(NB1):
                load_chunk_b(b3, 2)
            for kt in range(8, KB):
                load_xh(kt, 0)
            for b3 in range(NB1):
                load_chunk_f(b3, 0)
            for j in range(0, 4):
                load_xfh(j, 0)
            for b3 in range(NB1):
                for ci in range(1, len(CHF)):
                    load_chunk_f(b3, ci)
            for j in range(4, NP):
                load_xfh(j, 0)
            # Bias is tiny but descriptor-heavy; first needed at the first
            # eviction (~35us in).
            b_sb = bpool.tile([P, OT], mybir.dt.float32, name="b_sb")
            nc.sync.dma_start(b_sb[:], bv.ap())
            # second token halves
            for kt in range(KB):
                load_xh(kt, 1)
            for j in range(NP):
                load_xfh(j, 1)

            # ---- phase 1: blocks 0..2, two half-token passes ----
            for half in range(2):
                tts = (0, 1) if half == 0 else (2, 3)
                ps1 = {
                    (b3, tt): pspool.tile(
                        [P, TN], mybir.dt.float32, name=f"ps1_{b3}_{tt}", tag="ps"
                    )
                    for b3 in range(NB1)
                    for tt in tts
                }
                for kt in range(KB):
                    for b3 in range(NB1):
                        ci, off = KT2CHB[kt]
                        lhsT = wcB[(b3, ci)][:, kt - off, :]
                        for tt in tts:
                            nc.tensor.matmul(
                                ps1[(b3, tt)][:],
                                lhsT,
                                x_slice(kt, tt),
                                start=(kt == 0),
                                stop=False,
                            )
                for j in range(NP):
                    for b3 in range(NB1):
                        ci, off = J2CHF[j]
                        lhsT = wcF[(b3, ci)][:, j - off, :, :]
                        for tt in tts:
                            nc.tensor.matmul(
                                ps1[(b3, tt)][:],
                                lhsT,
                                xf_slice(j, tt),
                                start=False,
                                stop=(j == NP - 1),
                                perf_mode=DR,
                            )
                for b3 in range(NB1):
                    for tt in tts:
                        evict(ps1[(b3, tt)], b3, tt)

            # ---- phase 2: remaining blocks against the resident x ----
            for ot in range(NB1, OT):
                wb_sb = wbpool.tile(
                    [P, KB, P], mybir.dt.bfloat16, name=f"wb_{ot}", tag="wb"
                )
                nc.sync.dma_start(wb_sb[:], wPb.ap()[ot])
                wf_sb = wfpool.tile(
                    [P, NP, 2, P], mybir.dt.float8e4, name=f"wf_{ot}", tag="wf"
                )
                nc.sync.dma_start(wf_sb[:], wPf.ap()[ot])

                last_block = ot == OT - 1
                n_tt = TT - 1 if last_block else TT
                psums = [
                    pspool.tile(
                        [P, TN], mybir.dt.float32, name=f"ps_{ot}_{tt}", tag="ps"
                    )
                    for tt in range(n_tt)
                ]
                # all bf16 k-tiles (kt-outer, tt-inner), then all DoubleRow
                # pairs: 2 PE perf-mode transitions per block, and each
                # PSUM bank still finishes within ~2us of the block's end.
                for kt in range(KB):
                    lhsT = wb_sb[:, kt, :]
                    for tt in range(n_tt):
                        nc.tensor.matmul(
                            psums[tt][:],
                            lhsT,
                            x_slice(kt, tt),
                            start=(kt == 0),
                            stop=False,
                        )
                for j in range(NP):
                    lhsT = wf_sb[:, j, :, :]
                    for tt in range(n_tt):
                        nc.tensor.matmul(
                            psums[tt][:],
                            lhsT,
                            xf_slice(j, tt),
                            start=False,
                            stop=(j == NP - 1),
                            perf_mode=DR,
                        )
                for tt in range(n_tt):
                    evict(psums[tt], ot, tt)

                if last_block:
                    # The kernel's very last group (tt=3) is split into two
                    # half-width groups run sequentially, so the final
                    # evict+DMA chain (which nothing can overlap) covers
                    # 128KB instead of 256KB.
                    HN = TN // 2
                    for h in range(2):
                        psum = pspool.tile(
                            [P, HN], mybir.dt.float32, name=f"ps_l_{h}", tag="ps"
                        )
                        lo = 3 * TN + h * HN
                        tl = lo - TH  # within the B half-tiles
                        for kt in range(KB):
                            nc.tensor.matmul(
                                psum[:],
                                wb_sb[:, kt, :],
                                xB[kt][:, tl : tl + HN],
                                start=(kt == 0),
                                stop=False,
                            )
                        for j in range(NP):
                            nc.tensor.matmul(
                                psum[:],
                                wf_sb[:, j, :, :],
                                xfB[j][:, :, tl : tl + HN],
                                start=False,
                                stop=(j == NP - 1),
                                perf_mode=DR,
                            )
                        evict(psum, ot, None, lo=lo, n=HN)

    nc.compile()
    return nc


def prepare_inputs(x, weight, bias):
    """Host-side layout prep: transpose + cast per-core shards."""
    bf16 = ml_dtypes.bfloat16
    fp8 = ml_dtypes.float8_e4m3
    x = np.asarray(x, dtype=np.float32)
    weight = np.asarray(weight, dtype=np.float32)
    bias = np.asarray(bias, dtype=np.float32)
    w_bin = np.where(weight >= 0, np.float32(1.0), np.float32(-1.0))
    # wP[ot, p, kt, o] = sign(W)[ot*128+o, kt*128+p] — per-ot weight blocks,
    # contiguous along (kt, o) so block DMAs are contiguous per partition.
    wP_np = np.ascontiguousarray(w_bin.reshape(OT, P, KT, P).transpose(0, 3, 2, 1))
    wPb_np = np.ascontiguousarray(wP_np[:, :, :KB, :]).astype(bf16)
    wPf_np = np.ascontiguousarray(wP_np[:, :, KB:, :]).astype(fp8).reshape(
        OT, P, NP, 2, P
    )
    bv_np = np.ascontiguousarray(
        np.asarray(bias, dtype=np.float32).reshape(OT, P).T
    )  # [P, OT]; bias[o] at [o % 128, o // 128]
    in_maps = []
    for b in range(B):
        xT_np = np.ascontiguousarray(x[b].T)  # [in, tokens]
        in_maps.append(
            {
                "xTb": xT_np[: KB * P].astype(bf16),
                "xTf": xT_np[KB * P :].astype(fp8),
                "wPb": wPb_np,
                "wPf": wPf_np,
                "biasv": bv_np,
            }
        )
    return in_maps


def _ensure_ntff_hook_shim():
    """bass_utils' trace path imports antenv.axon_hooks, which some images
    lack; provide a working shim (or a None hook) so tracing never crashes."""
    import sys
    import types

    try:
        import antenv.axon_hooks  # noqa: F401

        return
    except ImportError:
        pass
    hook = None
    try:
        from trn_agent_boot.trn_boot import _ntff_profile_via_ctypes

        hook = _ntff_profile_via_ctypes("/opt/axon/libaxon_pjrt.so")
    except Exception:
        pass
    mod = types.ModuleType("antenv.axon_hooks")
    mod.get_axon_ntff_profile_hook = lambda: hook
    mod.set_axon_ntff_profile_hook = lambda h: None
    sys.modules["antenv.axon_hooks"] = mod
    try:
        import antenv

        antenv.axon_hooks = mod
    except ImportError:
        pass


def run(in_maps, trace=False, **kwargs):
    global _compiled_nc
    if _compiled_nc is None:
        _compiled_nc = build_program()
    _ensure_ntff_hook_shim()
    from concourse.bass_utils import run_bass_kernel_spmd

    return run_bass_kernel_spmd(
        _compiled_nc, in_maps, list(range(N_CORES)), trace=trace, **kwargs
    )


def kernel(x, weight, bias):
    res = run(prepare_inputs(x, weight, bias))
    out = np.empty((B, T, OUT_F), dtype=np.float32)
    for b in range(B):
        out[b] = res.results[b]["outT"].T
    return out
